# revision 63
# baseline (speedup 1.0000x reference)
"""BlockNet Trainium2 kernel: data-parallel over 8 NeuronCores.
54.2us predicted (TimelineSim; no NTFF hook in this container), from the
prior 57.3us via DMA-side restructuring; verified rel err 9.55e-3.

Additional second/third-pass findings:
- The kernel is LATENCY-bound in the blend pipeline, not throughput-bound:
  batching both j-groups into one [96,2,512] acc cuts ACT+DVE busy ~6us
  but balloons makespan to 63us (each blend then waits on 20 matmuls and
  PSUM depth halves). The OPPOSITE split (per-i accs, 5-matmul groups)
  also loses (57.0us): +5us ACT / +3.5us DVE of per-op overhead beats the
  latency gain. The pair/per-jg granularity is the measured optimum.
- Pair-0 weight DMAs must precede the ones-row DMAs: DMA_ENGINES is a
  single-slot resource, so their transfer otherwise queues behind slab
  chunk transfers (-0.9us).
- Tail floor: FC-bias -> out DMA -> end barriers costs a fixed ~3.2us
  (HWDGE slot 625 + DGE delay 650 + DMA-completion sem ~1.4us + barrier
  rounds). Splitting b4's accumulation across the last b3 blend is
  time-neutral (already hidden).
- Pinned dimensions (each perturbation measured neutral-or-worse):
  emission order (b2(4) early +3.3us; b3-before-b2(6) +3.8us), per-slot
  mode flips on any slot but 28, tail per-i splits (+0.5us), NW1=4
  (+5.0us), blend pool bufs 4->6 (exactly neutral).

Changes vs the 57.3us version (all DMA/startup side; the blend engine
assignment survived a large mode sweep unchanged except slot 28):
- slabA trimmed 97->79 rows (b1 jg0 only reads w<26): -433KB HBM.
- b1 jg1 weights split fixed(s,g)/dynamic(d) like jg0 and fused into one
  combined tile [79, jg, col, i2, kh]; fixed+dyn each load as ONE strided
  DMA with >=512B runs (below that the descriptor rate halves). HWDGE
  holds a 625ns single-slot overhead per DMA instruction, so instruction
  count matters as much as bytes: 30 -> 19 copies.
- pair 0's weights ride two small fused fixed+dyn DMAs so the first
  matmul starts ~3.5us instead of ~5.9us.
- Y-ones/ident/wfc/bfc moved to the SWDGE queue (tiny, 1-descriptor).
- blend slot 28 (b4, the serial tail) -> P-add + DVE 4x relu.

Engine-offload attempts that are ILLEGAL, for the record:
- PE identity-matmul add (start=False accumulate into the closed acc
  group): CoreSim rejects ("zero region has not been started"); on HW it
  corrupts NONDETERMINISTICALLY (passed 2 of 3 runs). Never trust a
  TimelineSim-only sweep: CoreSim-check AND numerically verify each pick.
- GPSIMD/Pool ops reading PSUM: walrus birverifier rejects (CoreSim
  does NOT catch this one).
- DMA evacuation of PSUM: dma_start asserts src in {SBUF, DRAM}.
So adds stay on DVE(1x-PSUM)/ACT+DVE(2x), muls on DVE (the PSUM operand
forces 1x); only relus can leave (Pool via SBUF, or DVE TSP 4x mode —
tensor_scalar/STT support 4x_2p all-SBUF-bf16, TensorTensor only 2x_1p).

Per core (batch NB=256), bf16 compute / f32 PSUM:
- Receptive-field DCE: the final 1x1 output depends only on
  x[:, :, 0:47, 0:47]; block1 computes (i,j) in [0,15)^2 only, block2
  [0,7)^2, block3 [0,3)^2, block4 (0,0). Host pre-casts xprep to bf16
  (bit-identical to the former SWDGE cast, halves slab HBM reads).
- One fused matmul per (block, j-group, i, kh): lhsT columns packed
  [s | g | d] in 32-aligned groups at partitions 0/32/64. All biases fold
  into the weight row hitting the all-ones row at the end of each input
  slab (kh=0 weights only). Block2's 6 output channels pad to 8 so Y2
  row-group boundaries stay 32-aligned.
- Blend y = relu(s + sigmoid(g)*d) on i-paired [96, 512] PSUM accs, the
  accumulation group closed at the last kh matmul before any read:
  ACT sigmoid -> DVE mul -> add via two alternating variants
  (P: DVE add reads s from PSUM directly; S: ACT evacuates s to SBUF,
  DVE adds in 2x bf16 mode) -> relu alternating ACT/DVE. This engine
  assignment and the 50/50 P:S ratio are a measured optimum; note the
  conservation law: exactly one of mul/add must pay the 1x-PSUM rate
  whatever the operand arrangement, so blend cost cannot be reduced by
  re-packing within the HW access-pattern rules below.
- Block1 jg0 weights split Fixed (s,g; loaded once per rotating tile) /
  Dynamic (d; one DMA per i-pair). Slab loads ride SWDGE (parallel
  queue); all castless DMAs ride HWDGE. Emission order is hand-tuned and
  SHARP (single-slot changes cost 5-6us): block1 tail pairs run as soon
  as their slab chunks land so the serial b2->b3->b4->FC chain starts
  early.

HW rules discovered by probe (walrus birverifier / CoreSim / HW crash):
- Engine AP partition ranges: start in {0,32,64,96}; must not cross the
  64-partition boundary unless starting at 0. Applies to ACT/DVE on both
  SBUF and PSUM. DMA partition bases are unconstrained.
- Matmul lhsT/rhs base in {0,32,64} (equal); out base per col-size rule.
- PSUM must not be read while its 2KB-bank accumulation group is open
  (start=True..stop=True); violating this hard-crashes the device.
- TensorTensor/STT: all SBUF operands must lie on identical partitions;
  PSUM operands and single-source copies (which may rebase) are exempt.
- GPSIMD tensor ops cost ~1.3us per [32,1024] op and share the Pool
  sequencer with SWDGE descriptor gen: not useful here.
"""
import numpy as np
import ml_dtypes

import concourse.bass as bass
import concourse.mybir as mybir
import concourse.bacc as bacc
import concourse.tile as tile
from concourse.bass_utils import run_bass_kernel_spmd

N_CORES = 8
NB = 256
BF16 = mybir.dt.bfloat16
F32 = mybir.dt.float32

# (cin, cout, k, s, oh_used, coutp)
CFG = [(3, 4, 5, 3, 15, 4), (4, 6, 3, 2, 7, 8), (6, 16, 3, 2, 3, 16),
       (16, 32, 3, 2, 1, 32)]
NJS = [[8, 7], [4, 3], [2, 1], [1]]
J0S = [[0, 8], [0, 4], [0, 2], [0]]
# rhs partition count (= full input tile rows, ones row last)
# b1 jg0 only reads w in [0,26) -> rows [0,78) + ones@78 (was 97)
KS = [[79, 70], [61, 61], [57, 57], [49]]
BIAS_ROW = [[78, 69], [60, 60], [56, 56], [48]]
W0 = [0, 24]                  # block1 slab w offsets

# Per-blend engine routing, keyed by blend emission index (0..28); value
# (mul, add, relu) per blend_pair's docstring. Missing index = baseline
# alternation (P/S by parity, relu act/dve by parity). NOTE: a PE
# identity-matmul add (start=False onto the closed acc group) was tried and
# is ILLEGAL — PSUM accumulation into a stopped group; CoreSim rejects it
# and HW corrupts nondeterministically. Every config change must be
# CoreSim-checked AND numerically verified, not just timed.
BLEND_MODES = {28: ("dve", "P", "dve")}

_CACHE = {}


def _prep_weights(inputs):
    arrs = {}
    for blk in range(4):
        cin, cout, k, st, oh, coutp = CFG[blk]
        Lfull = {0: 20, 1: 9, 2: 4, 3: 1}[blk]
        wu = np.asarray(inputs[f"w_uc{blk + 1}"], np.float32).reshape(
            Lfull * Lfull, cin * k * k, cout)
        bu = np.asarray(inputs[f"b_uc{blk + 1}"], np.float32)[0]
        wp = np.asarray(inputs[f"w_pc{blk + 1}"], np.float32)
        bp = np.asarray(inputs[f"b_pc{blk + 1}"], np.float32)
        wg = np.asarray(inputs[f"w_wl{blk + 1}"], np.float32)[0]
        bg = float(np.asarray(inputs[f"b_wl{blk + 1}"], np.float32)[0])
        cinp = 4 if blk == 1 else (8 if blk == 2 else cin)  # input row pitch

        def row_of(jg, w, c):
            if blk == 0:
                return (w - W0[jg]) * 3 + c
            return w * {1: 4, 2: 8, 3: 16}[blk] + c

        def fill(W, jg, i, kind, coloff):
            # W[row, col, kh]; cols pitch coutp within the group
            nj, j0 = NJS[blk][jg], J0S[blk][jg]
            brow = BIAS_ROW[blk][jg]
            for jt in range(nj):
                j = j0 + jt
                for kw in range(k):
                    w = st * j + kw
                    for c in range(cin):
                        r = row_of(jg, w, c)
                        kidx = c * k * k
                        for o in range(cout):
                            col = coloff + jt * coutp + o
                            for kh in range(k):
                                if kind == 's':
                                    v = wp[o, c, kh, kw]
                                elif kind == 'g':
                                    v = wg[c, kh, kw]
                                else:
                                    v = (wu[i * Lfull + j, kidx + kh * k + kw, o]
                                         - wp[o, c, kh, kw])
                                W[r, col, kh] = v
            for jt in range(nj):
                j = j0 + jt
                for o in range(cout):
                    col = coloff + jt * coutp + o
                    if kind == 's':
                        W[brow, col, 0] = bp[o]
                    elif kind == 'g':
                        W[brow, col, 0] = bg
                    else:
                        W[brow, col, 0] = bu[o, i, j] - bp[o]

        if blk == 0:
            # Combined per-jg weight arrays, tile layout [K, jg, col, i2, kh]:
            # fixed s/g cols (i-independent) load once per static tile, d cols
            # per pair — each as ONE strided DMA with contiguous runs >= 512B
            # (descriptor-rate 2x penalty below that; HWDGE pays 625ns/DMA).
            # jg1 (K=70) zero-padded to 79 rows.
            F = np.zeros((79, 2, 64, 5), np.float32)       # [K, i2, col, kh]
            F2 = np.zeros((70, 2, 64, 5), np.float32)
            for i2 in range(2):
                fill(F[:, i2], 0, 0, 's', 0)
                fill(F[:, i2], 0, 0, 'g', 32)
                fill(F2[:, i2], 1, 0, 's', 0)
                fill(F2[:, i2], 1, 0, 'g', 32)
            WF = np.zeros((79, 2, 64, 2, 5), np.float32)   # [K, jg, col, i2, kh]
            WF[:, 0] = F.transpose(0, 2, 1, 3)
            WF[0:70, 1] = F2.transpose(0, 2, 1, 3)
            arrs["w1f"] = WF.astype(ml_dtypes.bfloat16)
            D = np.zeros((8, 79, 2, 32, 5), np.float32)    # [p, K, i2, col, kh]
            D2 = np.zeros((8, 70, 2, 32, 5), np.float32)
            for p in range(8):
                for i2 in range(2):
                    i = 2 * p + i2
                    if i < 15:
                        fill(D[p, :, i2], 0, i, 'd', 0)
                        fill(D2[p, :, i2], 1, i, 'd', 0)
            WD = np.zeros((8, 79, 2, 32, 2, 5), np.float32)
            WD[:, :, 0] = D.transpose(0, 1, 3, 2, 4)
            WD[:, 0:70, 1] = D2.transpose(0, 1, 3, 2, 4)
            arrs["w1d"] = WD.astype(ml_dtypes.bfloat16)
            # pair 0 rides two small fused fixed+dyn DMAs (one per jg) so the
            # first matmuls start ~2.5us earlier than behind the big w1f load
            P0 = np.concatenate([WF, WD[0]], axis=2)   # [79, 2, 96, 2, 5]
            arrs["w1p0"] = P0.astype(ml_dtypes.bfloat16)
        else:
            K0 = KS[blk][0]
            njgs = len(NJS[blk])
            W = np.zeros((K0, oh, k, 96 * njgs), np.float32)
            for jg in range(njgs):
                for i in range(oh):
                    Wv = np.zeros((K0, 96, k), np.float32)
                    fill(Wv, jg, i, 's', 0)
                    fill(Wv, jg, i, 'g', 32)
                    fill(Wv, jg, i, 'd', 64)
                    W[:, i, :, 96 * jg:96 * (jg + 1)] = Wv.transpose(0, 2, 1)
            arrs[f"w{blk + 1}"] = W.astype(ml_dtypes.bfloat16)

    arrs["ones_slab"] = np.ones((1, 47, NB), np.float32).astype(ml_dtypes.bfloat16)
    arrs["ident"] = np.eye(32, dtype=np.float32).astype(ml_dtypes.bfloat16)
    arrs["wfc"] = np.asarray(inputs["fc_w"], np.float32).astype(ml_dtypes.bfloat16)
    arrs["bfc"] = np.asarray(inputs["fc_b"], np.float32).reshape(4, 1)
    return arrs


def _build():
    nc = bacc.Bacc("TRN2", target_bir_lowering=False, debug=False,
                   num_devices=N_CORES)
    xprep = nc.dram_tensor("xprep", [141, 47, NB], BF16, kind="ExternalInput").ap()
    w1f_d = nc.dram_tensor("w1f", [79, 2, 64, 2, 5], BF16, kind="ExternalInput").ap()
    w1d_d = nc.dram_tensor("w1d", [8, 79, 2, 32, 2, 5], BF16, kind="ExternalInput").ap()
    w1p0_d = nc.dram_tensor("w1p0", [79, 2, 96, 2, 5], BF16, kind="ExternalInput").ap()
    w2_d = nc.dram_tensor("w2", [61, 7, 3, 192], BF16, kind="ExternalInput").ap()
    w3_d = nc.dram_tensor("w3", [57, 3, 3, 192], BF16, kind="ExternalInput").ap()
    w4_d = nc.dram_tensor("w4", [49, 1, 3, 96], BF16, kind="ExternalInput").ap()
    ones_d = nc.dram_tensor("ones_slab", [1, 47, NB], BF16, kind="ExternalInput").ap()
    ident_d = nc.dram_tensor("ident", [32, 32], BF16, kind="ExternalInput").ap()
    wfc_d = nc.dram_tensor("wfc", [32, 4], BF16, kind="ExternalInput").ap()
    bfc_d = nc.dram_tensor("bfc", [4, 1], F32, kind="ExternalInput").ap()
    out_d = nc.dram_tensor("out", [4, NB], F32, kind="ExternalOutput").ap()

    with tile.TileContext(nc) as tc:
        import contextlib
        ctx = contextlib.ExitStack()
        with ctx:
            pconst = ctx.enter_context(tc.tile_pool(name="const", bufs=1))
            pslab = ctx.enter_context(tc.tile_pool(name="slab", bufs=1))
            pw = ctx.enter_context(tc.tile_pool(name="w", bufs=1))
            pg = ctx.enter_context(tc.tile_pool(name="g", bufs=4))
            pt = ctx.enter_context(tc.tile_pool(name="t", bufs=4))
            pu = ctx.enter_context(tc.tile_pool(name="u", bufs=4))
            pps = ctx.enter_context(tc.tile_pool(name="ps", bufs=7, space="PSUM"))
            ppsfc = ctx.enter_context(tc.tile_pool(name="psfc", bufs=1, space="PSUM"))

            slabA = pslab.tile([79, 47, NB], BF16, tag="slabA")
            slabB = pslab.tile([70, 47, NB], BF16, tag="slabB")
            Y1 = pslab.tile([61, 15, NB], BF16, tag="Y1")
            Y2 = pslab.tile([57, 7, NB], BF16, tag="Y2")
            Y3 = pslab.tile([49, 3, NB], BF16, tag="Y3")
            y4 = pslab.tile([32, NB], BF16, tag="y4")


            # slab loads (critical path): bf16, h-chunked on SWDGE
            for (ha, hb) in (((0, 8), (0, 8)), ((8, 20), (8, 24)),
                             ((20, 34), (24, 47)), ((34, 47), None)):
                nc.gpsimd.dma_start(slabA[0:78, ha[0]:ha[1], :],
                                    xprep[0:78, ha[0]:ha[1], :])
                if hb is not None:
                    nc.gpsimd.dma_start(slabB[0:69, hb[0]:hb[1], :],
                                        xprep[72:141, hb[0]:hb[1], :])

            # block1 W tiles [K, jg, col, i2, kh]: cols 0:64 fixed (s|g),
            # 64:96 dynamic (d); jg1 uses partitions 0:70 only
            NW1 = 3
            w1ts = []
            for b in range(NW1):
                t = pw.tile([79, 2, 96, 2, 5], BF16, tag=f"w1t{b}")
                w1ts.append(t)

            # pair-0 weights first: their transfers contend with slab chunks
            # for the single-slot DMA engine pipe and gate the first matmul;
            # the ones rows are tiny and still land long before needed.
            nc.sync.dma_start(w1ts[0][:, 0:1, :, :, :], w1p0_d[:, 0:1])
            nc.sync.dma_start(slabA[78:79, :, :], ones_d[:])
            nc.sync.dma_start(w1ts[0][:, 1:2, :, :, :], w1p0_d[:, 1:2])
            nc.sync.dma_start(slabB[69:70, :, :], ones_d[:])

            w2t = pw.tile([61, 7, 3, 192], BF16, tag="w2t")
            w3t = pw.tile([57, 3, 3, 192], BF16, tag="w3t")
            w4t = pw.tile([49, 1, 3, 96], BF16, tag="w4t")
            wfc_t = pconst.tile([32, 4], BF16, tag="wfc")
            bfc_t = pconst.tile([4, 1], F32, tag="bfc")
            ident_t = pconst.tile([32, 32], BF16, tag="ident")
            nc.gpsimd.dma_start(ident_t[:], ident_d[:])

            blend_ctr = [0]
            pending = []

            def flush_pending():
                for f in pending:
                    f()
                pending.clear()

            def emit_relu(dst, src, eng):
                if eng == "act":
                    nc.scalar.activation(dst, src,
                                         mybir.ActivationFunctionType.Relu)
                elif eng == "dve":
                    nc.vector.tensor_scalar_max(dst, src, 0.0)
                else:
                    nc.gpsimd.tensor_scalar_max(dst, src, 0.0)

            def blend_pair(acc, njcs, dsts, ni):
                """Blend y = relu(s + sigmoid(g)*d) for one i-pair/j-group.
                acc [96, 1, 512]: rows [s32|g32|d32(pad)]. Engine route per
                BLEND_MODES[n] = (mul, add, relu):
                  mul: 'dve' TT from PSUM (1x) | 'pool' gpsimd
                  add: 'P' DVE TT from PSUM | 'S' ACT evacuates s + DVE 2x TT
                       | 'S4' ACT evacuates s + DVE 4x STT | 'PL' gpsimd add
                       from PSUM
                  relu: 'act' | 'dve' (4x TSP) | 'pool'."""
                n = blend_ctr[0]
                blend_ctr[0] += 1
                fd = 256 * ni
                njg = len(njcs)
                flush_pending()
                mode = BLEND_MODES.get(n)
                meng, add_variant, reng = mode if mode is not None else (
                    "dve", "P" if n % 2 == 1 else "S", None)
                if meng == "V":
                    # Evacuate s and d PSUM->SBUF via SWDGE (casts f32->bf16
                    # in the DMA engine, which is idle mid-kernel); the whole
                    # DVE chain then runs in 2x/4x all-SBUF modes.
                    sd_t = pt.tile([64, 2, 512], BF16, tag="sd")
                    nc.gpsimd.dma_start(sd_t[0:32, 0:njg, 0:fd],
                                        acc[0:32, 0:njg, 0:fd])
                    nc.gpsimd.dma_start(sd_t[32:64, 0:njg, 0:fd],
                                        acc[64:96, 0:njg, 0:fd])
                    g_t = pg.tile([32, 2, 512], BF16, tag="g")
                    nc.scalar.activation(g_t[:, 0:njg, 0:fd],
                                         acc[32:64, 0:njg, 0:fd],
                                         mybir.ActivationFunctionType.Sigmoid)
                    t_t = pt.tile([32, 2, 512], BF16, tag="t")
                    nc.vector.tensor_mul(t_t[:, 0:njg, 0:fd],
                                         sd_t[32:64, 0:njg, 0:fd],
                                         g_t[:, 0:njg, 0:fd])
                    u_t = pu.tile([32, 2, 512], BF16, tag="u")
                    nc.vector.scalar_tensor_tensor(
                        u_t[:, 0:njg, 0:fd], t_t[:, 0:njg, 0:fd], 1.0,
                        sd_t[0:32, 0:njg, 0:fd],
                        mybir.AluOpType.mult, mybir.AluOpType.add)
                    for jg, (njc, dst) in enumerate(zip(njcs, dsts)):
                        if dst is not None:
                            emit_relu(dst, u_t[0:njc, jg, 0:fd], reng)
                    return
                g_t = pg.tile([32, 2, 512], BF16, tag="g")
                nc.scalar.activation(g_t[:, 0:njg, 0:fd], acc[32:64, 0:njg, 0:fd],
                                     mybir.ActivationFunctionType.Sigmoid)
                t_t = pt.tile([32, 2, 512], BF16, tag="t")
                mul_eng = nc.vector if meng == "dve" else nc.gpsimd
                mul_eng.tensor_mul(t_t[:, 0:njg, 0:fd], acc[64:96, 0:njg, 0:fd],
                                   g_t[:, 0:njg, 0:fd])
                u_t = pu.tile([32, 2, 512], BF16, tag="u")
                if add_variant == "P":
                    # P-variant: add straight from PSUM (in0 is partition-
                    # exempt); saves the s-evacuation op entirely
                    nc.vector.tensor_add(u_t[:, 0:njg, 0:fd],
                                         acc[0:32, 0:njg, 0:fd],
                                         t_t[:, 0:njg, 0:fd])
                elif add_variant == "PL":
                    nc.gpsimd.tensor_add(u_t[:, 0:njg, 0:fd],
                                         acc[0:32, 0:njg, 0:fd],
                                         t_t[:, 0:njg, 0:fd])
                else:
                    s_t = pu.tile([32, 2, 512], BF16, tag="s")
                    nc.scalar.activation(s_t[:, 0:njg, 0:fd],
                                         acc[0:32, 0:njg, 0:fd],
                                         mybir.ActivationFunctionType.Copy)
                    if add_variant == "S4":
                        # STT (t*1.0)+s: all-SBUF bf16 packed -> 4x DVE mode
                        nc.vector.scalar_tensor_tensor(
                            u_t[:, 0:njg, 0:fd], t_t[:, 0:njg, 0:fd], 1.0,
                            s_t[:, 0:njg, 0:fd],
                            mybir.AluOpType.mult, mybir.AluOpType.add)
                    else:
                        nc.vector.tensor_add(u_t[:, 0:njg, 0:fd],
                                             t_t[:, 0:njg, 0:fd],
                                             s_t[:, 0:njg, 0:fd])
                for jg, (njc, dst) in enumerate(zip(njcs, dsts)):
                    if dst is None:
                        continue
                    src = u_t[0:njc, jg, 0:fd]
                    r = reng if reng is not None else (
                        "act" if (n + jg) % 2 == 0 else "dve")
                    emit_relu(dst, src, r)

            # ---- pair emitters ----
            def emit_b1(p):
                # block 1: 15 i in 8 pairs, jg0 (j 0..7) + jg1 (j 8..14)
                ni = 2 if p < 7 else 1
                wt = w1ts[p % NW1]
                if p > 0:
                    if p < NW1:
                        nc.sync.dma_start(wt[:, :, 0:64, :, :], w1f_d[:])
                    nc.sync.dma_start(wt[:, :, 64:96, :, :], w1d_d[p])
                for jg, (slab, K, njc, r0) in enumerate((
                        (slabA, 79, 32, 0), (slabB, 70, 28, 32))):
                    acc = pps.tile([96, 1, 512], F32, tag="acc")
                    for i2 in range(ni):
                        i = 2 * p + i2
                        for kh in range(5):
                            nc.tensor.matmul(
                                acc[0:96, 0, 256 * i2:256 * (i2 + 1)],
                                wt[0:K, jg, :, i2, kh],
                                slab[:, 3 * i + kh, :],
                                start=(i2 == 0 and kh == 0),
                                stop=(i2 == ni - 1 and kh == 4))
                    blend_pair(acc, (njc,),
                               (Y1[r0:r0 + njc, 2 * p:2 * p + ni, :],), ni)

            def emit_small(blk, wtile, src, dstY, p0, ni):
                cin, cout, k, st, oh, coutp = CFG[blk]
                njcs = [nj * coutp for nj in NJS[blk]]
                roffs = np.cumsum([0] + njcs)[:-1]
                for jg, njc in enumerate(njcs):
                    K = KS[blk][jg]
                    acc = pps.tile([96, 1, 512], F32, tag="acc")
                    for i2 in range(ni):
                        i = p0 + i2
                        for kh in range(k):
                            nc.tensor.matmul(
                                acc[0:96, 0, 256 * i2:256 * (i2 + 1)],
                                wtile[0:K, i, kh, 96 * jg:96 * (jg + 1)],
                                src[0:K, st * i + kh, :],
                                start=(i2 == 0 and kh == 0),
                                stop=(i2 == ni - 1 and kh == k - 1))
                    if dstY is y4:
                        dst = y4[0:32, :]
                    else:
                        dst = dstY[roffs[jg]:roffs[jg] + njc, p0:p0 + ni, :]
                    blend_pair(acc, (njc,), (dst,), ni)

            # hand-interleaved schedule: block1 tail pairs (p6/p7) run as
            # soon as their slab chunks land so the serial b2->b3->b4 tail
            # chain starts early; early b2 pairs fill ACT/DVE meanwhile.
            emit_b1(0)
            emit_b1(1)
            nc.gpsimd.dma_start(Y1[60:61, :, :], ones_d[0:1, 0:15, :])
            nc.gpsimd.dma_start(Y2[56:57, :, :], ones_d[0:1, 0:7, :])
            nc.gpsimd.dma_start(Y3[48:49, :, :], ones_d[0:1, 0:3, :])
            emit_b1(2)
            nc.sync.dma_start(w2t[:], w2_d[:])
            emit_small(1, w2t, Y1, Y2, 0, 2)       # needs b1 i<=3
            emit_b1(3)
            nc.sync.dma_start(w3t[:], w3_d[:])
            emit_b1(4)
            nc.sync.dma_start(w4t[:], w4_d[:])
            nc.sync.dma_start(wfc_t[:], wfc_d[:])
            nc.sync.dma_start(bfc_t[:], bfc_d[:])
            emit_small(1, w2t, Y1, Y2, 2, 2)       # needs b1 i<=7
            emit_b1(5)
            emit_b1(6)
            emit_b1(7)
            emit_small(1, w2t, Y1, Y2, 4, 2)       # needs b1 i<=11
            emit_small(1, w2t, Y1, Y2, 6, 1)       # needs b1 i<=14
            emit_small(2, w3t, Y2, Y3, 0, 2)       # needs b2 i<=5
            emit_small(2, w3t, Y2, Y3, 2, 1)       # needs b2 i<=6
            emit_small(3, w4t, Y3, y4, 0, 1)       # needs b3 all

            # ---- FC ----
            flush_pending()
            accfc = ppsfc.tile([4, NB], F32, tag="accfc")
            nc.tensor.matmul(accfc[:], wfc_t[:], y4[:], start=True, stop=True)
            out_t = pconst.tile([4, NB], F32, tag="outt")
            nc.scalar.activation(out_t[:], accfc[:],
                                 mybir.ActivationFunctionType.Identity,
                                 bias=bfc_t[:])
            nc.sync.dma_start(out_d[:], out_t[:])

    nc.compile()
    return nc


def _make_in_maps(inputs):
    warrs = _prep_weights(inputs)
    x = np.asarray(inputs["x"], np.float32)
    in_maps = []
    for ci in range(N_CORES):
        xc = x[ci * NB:(ci + 1) * NB, :, 0:47, 0:47]        # [256,3,47,47]
        xprep = np.ascontiguousarray(
            xc.transpose(3, 1, 2, 0)).reshape(141, 47, NB).astype(
            ml_dtypes.bfloat16)                             # [(w,c),h,b]
        m = {"xprep": xprep}
        m.update(warrs)
        in_maps.append(m)
    return in_maps


def kernel(**inputs):
    if "nc" not in _CACHE:
        _CACHE["nc"] = _build()
    nc = _CACHE["nc"]
    in_maps = _make_in_maps(inputs)
    res = run_bass_kernel_spmd(nc, in_maps, core_ids=list(range(N_CORES)))
    out = np.concatenate([res.results[c]["out"].T for c in range(N_CORES)], axis=0)
    return out.astype(np.float32)

